# revision 1
# baseline (speedup 1.0000x reference)
"""GQA (16 query heads, 4 KV groups) forward kernel for 8 Trainium2 NeuronCores.

Sharding: core = (batch b in 0..1) x (kv-group g in 0..3).  Each core owns one
batch element and one whole KV group (4 query heads), computing the output
slice out[b, :, g*512:(g+1)*512].

Per-core plan (all matmul inputs bf16, fp32 PSUM accumulation):
  - x^T loaded straight from DRAM via xbar DMA-transpose (bf16).
  - Q^T [128, T] per head, K^T [128, T], V^T -> V natural [T, 128] on PE.
  - Attention in transposed-score layout: S^T(kb, qc) = K_blk @ Q^T_chunk
    ([128 k x 512 q] blocks), exp on ACT (scale 1/sqrt(hd) folded in), causal
    mask via gpsimd affine_select (zeroing), softmax denominators accumulated
    on DVE + reduced via PE transpose, P@V accumulated in PSUM with V natural
    as the stationary operand, final PE transpose + 1/sum scaling.
"""

import sys

if "/opt/trn_rl_repo" not in sys.path:
    sys.path.insert(0, "/opt/trn_rl_repo")

import ml_dtypes
import numpy as np

import concourse.bass as bass
import concourse.mybir as mybir
import concourse.tile as tile
from concourse import bacc
from concourse.bass_utils import run_bass_kernel_spmd
from concourse.masks import make_identity

B, T, C = 2, 2048, 2048
HEADS, GROUPS = 16, 4
HD = C // HEADS          # 128 head dim
H2G = HEADS // GROUPS    # 4 query heads per group
DG = H2G * HD            # 512 output cols per core
DKV = HD                 # 128 kv dim per group
NCT = C // 128           # 16 contraction tiles
NQC = T // 512           # 4 query chunks
NKB = T // 128           # 16 key blocks
SCALE = HD ** -0.5

F32 = mybir.dt.float32
BF16 = mybir.dt.bfloat16


def _body(tc, xb, wqt, wkt, wvt, out_d):
    nc = tc.nc
    act_exp = mybir.ActivationFunctionType.Exp
    axis_x = mybir.AxisListType.X
    alu_add = mybir.AluOpType.add
    is_ge = mybir.AluOpType.is_ge

    with (
        tc.tile_pool(name="const", bufs=1) as cpool,
        tc.tile_pool(name="data", bufs=1) as data,
    ):
        id_b = cpool.tile([128, 128], BF16)
        make_identity(nc, id_b)
        id_f = cpool.tile([128, 128], F32)
        make_identity(nc, id_f)

        xT = data.tile([128, NCT, T], BF16)    # x^T: [c%128, c//128, t]
        wq = data.tile([128, NCT, DG], BF16)   # Wq^T tiles [c%128, c//128, d]
        wk = data.tile([128, NCT, DKV], BF16)
        wv = data.tile([128, NCT, DKV], BF16)
        kT = data.tile([128, T], BF16)         # K^T: [d, t]
        vn = data.tile([128, NKB, DKV], BF16)  # V natural: [t%128, t//128, d]

        for ci in range(NCT):
            nc.sync.dma_start(out=xT[:, ci, :], in_=xb[ci * 128:(ci + 1) * 128, :])
            nc.sync.dma_start(out=wq[:, ci, :], in_=wqt[ci * 128:(ci + 1) * 128, :])
            nc.sync.dma_start(out=wk[:, ci, :], in_=wkt[ci * 128:(ci + 1) * 128, :])
            nc.sync.dma_start(out=wv[:, ci, :], in_=wvt[ci * 128:(ci + 1) * 128, :])

        # ---- K/V projections ----
        with (
            tc.tile_pool(name="proj_ps", bufs=4, space="PSUM") as pp,
            tc.tile_pool(name="vt_stage", bufs=1) as vstg,
            tc.tile_pool(name="vt_ps", bufs=2, space="PSUM") as vtp,
        ):
            for qc in range(NQC):
                ps = pp.tile([128, 512], F32, tag="proj")
                for ci in range(NCT):
                    nc.tensor.matmul(
                        ps[:], wk[:, ci, :], xT[:, ci, qc * 512:(qc + 1) * 512],
                        start=(ci == 0), stop=(ci == NCT - 1),
                    )
                nc.vector.tensor_copy(kT[:, qc * 512:(qc + 1) * 512], ps[:])
            vT = vstg.tile([128, T], BF16)
            for qc in range(NQC):
                ps = pp.tile([128, 512], F32, tag="proj")
                for ci in range(NCT):
                    nc.tensor.matmul(
                        ps[:], wv[:, ci, :], xT[:, ci, qc * 512:(qc + 1) * 512],
                        start=(ci == 0), stop=(ci == NCT - 1),
                    )
                nc.vector.tensor_copy(vT[:, qc * 512:(qc + 1) * 512], ps[:])
            for tb in range(NKB):
                pt = vtp.tile([128, 128], BF16, tag="vtp")
                nc.tensor.transpose(pt[:], vT[:, tb * 128:(tb + 1) * 128], id_b)
                nc.vector.tensor_copy(vn[:, tb, :], pt[:])

        # ---- attention, with per-chunk Q^T production interleaved ----
        with (
            tc.tile_pool(name="qt_ps", bufs=2, space="PSUM") as qtp,
            tc.tile_pool(name="st_ps", bufs=2, space="PSUM") as stp,
            tc.tile_pool(name="pv_ps", bufs=1, space="PSUM") as pvp,
            tc.tile_pool(name="qt_sb", bufs=2) as qtsb,
            tc.tile_pool(name="ex_sb", bufs=10) as expool,
            tc.tile_pool(name="sum_sb", bufs=2) as sump,
            tc.tile_pool(name="o_sb", bufs=2) as outp,
            tc.tile_pool(name="r_sb", bufs=2) as rsp,
        ):
            def make_qt(qc):
                # Q^T chunk [d, h, 512] for all 4 heads at this q-chunk
                qt = qtsb.tile([128, H2G, 512], BF16, tag="qt", name=f"qt{qc}")
                for h in range(H2G):
                    ps = qtp.tile([128, 512], F32, tag="qtp", name=f"qtp{qc}_{h}")
                    for ci in range(NCT):
                        nc.tensor.matmul(
                            ps[:],
                            wq[:, ci, h * 128:(h + 1) * 128],
                            xT[:, ci, qc * 512:(qc + 1) * 512],
                            start=(ci == 0), stop=(ci == NCT - 1),
                        )
                    nc.vector.tensor_copy(qt[:, h, :], ps[:])
                return qt

            qt_next = make_qt(0)
            for qc in range(NQC):
                qt = qt_next
                pv = [
                    pvp.tile([128, 512], F32, tag=f"pv{h}", name=f"pv{h}")
                    for h in range(H2G)
                ]
                sums = [
                    sump.tile([128, 512], F32, tag=f"sum{h}", name=f"sum{h}")
                    for h in range(H2G)
                ]
                osb = outp.tile([128, 4, DG], F32, tag="osb")
                nkb = 4 * qc + 4
                for kb in range(nkb):
                    diag = kb >= 4 * qc
                    exs = []
                    # all 4 scores matmuls share the stationary K^T block
                    for h in range(H2G):
                        st = stp.tile([128, 512], F32, tag="st", name=f"st{h}")
                        nc.tensor.matmul(
                            st[:],
                            kT[:, kb * 128:(kb + 1) * 128],
                            qt[:, h, :],
                            start=True, stop=True,
                        )
                        ex = expool.tile([128, 512], BF16, tag="ex", name=f"ex{h}")
                        nc.scalar.activation(ex[:], st[:], act_exp, scale=SCALE)
                        if diag:
                            # keep where (qc*512 + f) >= (kb*128 + p)
                            nc.gpsimd.affine_select(
                                out=ex[:], in_=ex[:],
                                compare_op=is_ge,
                                fill=0.0,
                                base=qc * 512 - kb * 128,
                                pattern=[[1, 512]],
                                channel_multiplier=-1,
                            )
                        if kb == 0:
                            nc.vector.tensor_copy(sums[h][:], ex[:])
                        else:
                            nc.vector.tensor_add(sums[h][:], sums[h][:], ex[:])
                        exs.append(ex)
                    # all 4 PV matmuls share the stationary V block
                    for h in range(H2G):
                        nc.tensor.matmul(
                            pv[h][:], vn[:, kb, :], exs[h][:],
                            start=(kb == 0), stop=(kb == nkb - 1),
                        )
                # produce next chunk's Q^T before this chunk's wrap-up so PE
                # stays dense while DVE finishes the softmax denominators
                if qc + 1 < NQC:
                    qt_next = make_qt(qc + 1)
                for h in range(H2G):
                    rsum = rsp.tile([128, 4], F32, tag="rsum")
                    rinv = rsp.tile([128, 4], F32, tag="rinv")
                    for j in range(4):
                        tp = stp.tile([128, 128], F32, tag="st", name=f"tr{j}")
                        nc.tensor.transpose(tp[:], sums[h][:, j * 128:(j + 1) * 128], id_f)
                        nc.vector.tensor_reduce(rsum[:, j:j + 1], tp[:], axis=axis_x, op=alu_add)
                    nc.vector.reciprocal(rinv[:], rsum[:])
                    ot = outp.tile([128, 512], F32, tag="ot")
                    nc.vector.tensor_copy(ot[:], pv[h][:])
                    for j in range(4):
                        tp2 = stp.tile([128, 128], F32, tag="st", name=f"tr2{j}")
                        nc.tensor.transpose(tp2[:], ot[:, j * 128:(j + 1) * 128], id_f)
                        nc.vector.tensor_scalar_mul(
                            osb[:, j, h * 128:(h + 1) * 128], tp2[:], rinv[:, j:j + 1]
                        )
                # One store per q-chunk on SWDGE: single DMA per queue keeps
                # each store at a single sync wait (walrus descriptor limit).
                o_view = out_d[qc * 512:(qc + 1) * 512, :].rearrange(
                    "(j p) d -> p j d", p=128
                )
                nc.gpsimd.dma_start(out=o_view, in_=osb[:, :, :])


def build_nc():
    # Bacc (not raw Bass): its finalize passes split multi-sem waits
    # (move_matmul_waits_to_ldweights / generate_event_semaphores) to meet the
    # 1-wait-per-instruction hardware constraint walrus enforces.
    nc = bacc.Bacc("TRN2", target_bir_lowering=False)
    # xb is x[b] pre-transposed on the host: [C, T] bf16
    xb = nc.declare_dram_parameter("xb", [C, T], BF16, isOutput=False)
    wqt = nc.declare_dram_parameter("wqt", [C, DG], BF16, isOutput=False)
    wkt = nc.declare_dram_parameter("wkt", [C, DKV], BF16, isOutput=False)
    wvt = nc.declare_dram_parameter("wvt", [C, DKV], BF16, isOutput=False)
    out_d = nc.declare_dram_parameter("out", [T, DG], F32, isOutput=True)
    with tile.TileContext(nc) as tc:
        _body(tc, xb, wqt, wkt, wvt, out_d)
    nc.compile()
    return nc


def make_in_maps(x, Wq, Wk, Wv):
    bf = ml_dtypes.bfloat16
    in_maps = []
    for b in range(B):
        xb = np.ascontiguousarray(x[b].T).astype(bf)
        for g in range(GROUPS):
            in_maps.append({
                "xb": xb,
                "wqt": np.ascontiguousarray(Wq[g * DG:(g + 1) * DG].T).astype(bf),
                "wkt": np.ascontiguousarray(Wk[g * DKV:(g + 1) * DKV].T).astype(bf),
                "wvt": np.ascontiguousarray(Wv[g * DKV:(g + 1) * DKV].T).astype(bf),
            })
    return in_maps


def assemble(results):
    out = np.empty((B, T, C), np.float32)
    for i, res in enumerate(results):
        b, g = divmod(i, GROUPS)
        out[b, :, g * DG:(g + 1) * DG] = res["out"]
    return out


def run(x, Wq, Wk, Wv, **spmd_kwargs):
    nc = build_nc()
    in_maps = make_in_maps(x, Wq, Wk, Wv)
    return run_bass_kernel_spmd(nc, in_maps, list(range(8)), **spmd_kwargs)


def kernel(x, Wq, Wk, Wv):
    return assemble(run(x, Wq, Wk, Wv).results)



# revision 3
# speedup vs baseline: 1.1183x; 1.1183x over previous
"""GQA (16 query heads, 4 KV groups) forward kernel for 8 Trainium2 NeuronCores.

Sharding: core = (batch b in 0..1) x (kv-group g in 0..3).  Each core owns one
batch element and one whole KV group (4 query heads), computing the output
slice out[b, :, g*512:(g+1)*512].

Per-core plan (all matmul inputs fp16, fp32 PSUM accumulation):
  - x^T arrives from DRAM packed by t-chunk so K/V projections start as soon
    as the first 2.1MB chunk lands (DMA order: wk, wv, x0, wq, x1..x3).
  - K^T produced directly ([d, t], stationary Wk); V produced in natural
    layout ([t, d]) via x-stationary matmuls - no PE transposes anywhere.
  - Attention in transposed-score layout, two heads per pass so ACT exp and
    DVE sum-adds run on paired [128, 2, 512] tiles (half the instruction
    overhead).  Causal mask via gpsimd affine_select; exp restricted to the
    unmasked column range on diagonal blocks.
  - Softmax denominators via ones-vector matmul (partition reduction on PE),
    reciprocal on DVE, broadcast via gpsimd partition_broadcast, one DVE
    multiply to normalize - output stays in [d, q] layout and the host
    transposes it back during assemble.
  - Q^T chunks and later K/V projection chunks are interleaved as "filler"
    PE work inside the attention kb-loops so the PE never waits on ACT.
"""

import sys

if "/opt/trn_rl_repo" not in sys.path:
    sys.path.insert(0, "/opt/trn_rl_repo")

import numpy as np

import concourse.bass as bass
import concourse.mybir as mybir
import concourse.tile as tile
from concourse import bacc
from concourse.bass_utils import run_bass_kernel_spmd

B, T, C = 2, 2048, 2048
HEADS, GROUPS = 16, 4
HD = C // HEADS          # 128 head dim
H2G = HEADS // GROUPS    # 4 query heads per group
DG = H2G * HD            # 512 output cols per core
DKV = HD                 # 128 kv dim per group
NCT = C // 128           # 16 contraction tiles
NQC = T // 512           # 4 query chunks (= t chunks)
NKB = T // 128           # 16 key blocks
SCALE = HD ** -0.5

F32 = mybir.dt.float32
FP16 = mybir.dt.float16


def _body(tc, xb, wqt, wkt, wvt, out_d):
    nc = tc.nc
    act_exp = mybir.ActivationFunctionType.Exp
    is_ge = mybir.AluOpType.is_ge
    alu_mult = mybir.AluOpType.mult

    with (
        tc.tile_pool(name="const", bufs=1) as cpool,
        tc.tile_pool(name="data", bufs=1) as data,
        tc.tile_pool(name="qt_sb", bufs=2) as qtsb,
        tc.tile_pool(name="ex_sb", bufs=6) as expool,
        tc.tile_pool(name="sum_sb", bufs=3) as sump,
        tc.tile_pool(name="o_sb", bufs=2) as outp,
        tc.tile_pool(name="rv_sb", bufs=4) as rvp,
        tc.tile_pool(name="rb_sb", bufs=4) as rbp,
        tc.tile_pool(name="pv_ps", bufs=1, space="PSUM") as pvp,
        tc.tile_pool(name="st_ps", bufs=2, space="PSUM") as stp,
        tc.tile_pool(name="mi_ps", bufs=2, space="PSUM") as mip,
    ):
        ones_p = cpool.tile([128, 1], FP16)
        nc.vector.memset(ones_p[:], 1.0)

        xT = data.tile([128, NQC, NCT, 512], FP16)  # [c%128, tchunk, ci, t]
        wq = data.tile([128, NCT, DG], FP16)        # Wq^T tiles [c%128, ci, d]
        wk = data.tile([128, NCT, DKV], FP16)
        wv = data.tile([128, NCT, DKV], FP16)
        kT = data.tile([128, NQC, 512], FP16)       # K^T: [d, tchunk, t]
        vn = data.tile([128, NKB, DKV], FP16)       # V natural: [t%128, kb, d]

        # ---- input DMAs, in priority order ----
        nc.sync.dma_start(
            out=wk[:], in_=wkt.rearrange("(ci p) d -> p ci d", p=128))
        nc.sync.dma_start(
            out=wv[:], in_=wvt.rearrange("(ci p) d -> p ci d", p=128))
        for ci in range(NCT):
            nc.sync.dma_start(out=xT[:, 0, ci, :], in_=xb[0, ci])
        for c4 in range(4):
            nc.sync.dma_start(
                out=wq[:, c4 * 4:(c4 + 1) * 4, :],
                in_=wqt.rearrange("(ci p) d -> p ci d", p=128)[
                    :, c4 * 4:(c4 + 1) * 4, :])
        for tcx in range(1, NQC):
            for ci in range(NCT):
                nc.sync.dma_start(out=xT[:, tcx, ci, :], in_=xb[tcx, ci])

        # ---- projection chunk emitters (each ~1-4us of PE work) ----
        def k_chunk(tcx):
            ps = mip.tile([128, 512], F32, tag="mi", name=f"kp{tcx}")
            for ci in range(NCT):
                nc.tensor.matmul(
                    ps[:], wk[:, ci, :], xT[:, tcx, ci, :],
                    start=(ci == 0), stop=(ci == NCT - 1))
            nc.scalar.copy(kT[:, tcx, :], ps[:])

        def v_chunk(tcx, tb):
            ps = mip.tile([128, 128], F32, tag="mi", name=f"vp{tcx}_{tb}")
            for ci in range(NCT):
                nc.tensor.matmul(
                    ps[:], xT[:, tcx, ci, tb * 128:(tb + 1) * 128],
                    wv[:, ci, :],
                    start=(ci == 0), stop=(ci == NCT - 1))
            nc.scalar.copy(vn[:, tcx * 4 + tb, :], ps[:])

        qt_tiles = {}

        def q_chunk(qc, h):
            if qc not in qt_tiles:
                qt_tiles[qc] = qtsb.tile(
                    [128, H2G, 512], FP16, tag="qt", name=f"qt{qc}")
            qt = qt_tiles[qc]
            ps = mip.tile([128, 512], F32, tag="mi", name=f"qp{qc}_{h}")
            for ci in range(NCT):
                nc.tensor.matmul(
                    ps[:], wq[:, ci, h * 128:(h + 1) * 128],
                    xT[:, qc, ci, :],
                    start=(ci == 0), stop=(ci == NCT - 1))
            nc.scalar.copy(qt[:, h, :], ps[:])
            return qt

        # filler queue: (stage, deadline_iter_within_pass(stage,0), emit_fn)
        fillers = []
        for s in range(1, NQC):
            for h in range(H2G):
                fillers.append((s, 0, lambda s=s, h=h: q_chunk(s, h)))
            fillers.append((s, max(0, 4 * s - 2), lambda s=s: k_chunk(s)))
            for tb in range(4):
                fillers.append(
                    (s, max(0, 4 * s + tb - 2), lambda s=s, tb=tb: v_chunk(s, tb)))
        fillers.reverse()  # pop() from the front

        def drain_fillers(stage, itr):
            while fillers and (fillers[-1][0], fillers[-1][1]) <= (stage, itr):
                fillers.pop()[2]()

        def pop_filler():
            if fillers:
                fillers.pop()[2]()

        # ---- startup: KV chunk 0, Q^T chunk 0 ----
        k_chunk(0)
        for tb in range(4):
            v_chunk(0, tb)
        for h in range(H2G):
            q_chunk(0, h)

        # ---- attention: two heads per pass ----
        for qc in range(NQC):
            drain_fillers(qc, 0)
            qt = qt_tiles[qc]
            nkb = 4 * qc + 4
            for hp in range(2):
                sums = sump.tile([128, 2, 512], FP16, tag="sums",
                                 name=f"sums{qc}_{hp}")
                pv = [
                    pvp.tile([128, 512], F32, tag=f"pv{hh}", name=f"pv{qc}_{hp}_{hh}")
                    for hh in range(2)
                ]
                for kb in range(nkb):
                    if hp == 0:
                        drain_fillers(qc, kb)
                    diag = kb >= 4 * qc
                    base = (kb - 4 * qc) * 128 if diag else 0
                    st = stp.tile([128, 2, 512], F32, tag="st",
                                  name=f"st{qc}_{hp}_{kb}")
                    kblk = kT[:, kb // 4, (kb % 4) * 128:(kb % 4 + 1) * 128]
                    for hh in range(2):
                        nc.tensor.matmul(
                            st[:, hh, :], kblk, qt[:, 2 * hp + hh, :],
                            start=True, stop=True)
                    ex = expool.tile([128, 2, 512], FP16, tag="ex",
                                     name=f"ex{qc}_{hp}_{kb}")
                    nc.scalar.activation(
                        ex[:, :, base:], st[:, :, base:], act_exp, scale=SCALE)
                    if diag:
                        # columns < base are fully masked; [base, base+128)
                        # is the triangular boundary strip; >= base+128 kept.
                        if base > 0:
                            nc.gpsimd.memset(ex[:, :, :base], 0.0)
                        for hh in range(2):
                            nc.gpsimd.affine_select(
                                out=ex[:, hh, base:base + 128],
                                in_=ex[:, hh, base:base + 128],
                                compare_op=is_ge,
                                fill=0.0,
                                base=0,
                                pattern=[[1, 128]],
                                channel_multiplier=-1,
                            )
                    if kb == 0:
                        nc.vector.tensor_copy(sums[:], ex[:])
                    else:
                        nc.vector.tensor_add(sums[:], sums[:], ex[:])
                    for hh in range(2):
                        nc.tensor.matmul(
                            pv[hh][:], vn[:, kb, :], ex[:, hh, :],
                            start=(kb == 0), stop=(kb == nkb - 1))
                    if kb % 2 == 1 and hp == 1:
                        pop_filler()
                # ---- wrap-up: denominators + normalize + store ----
                o_sb = outp.tile([128, 2, 512], F32, tag="o",
                                 name=f"o{qc}_{hp}")
                for hh in range(2):
                    rs = mip.tile([1, 512], F32, tag="mi",
                                  name=f"rs{qc}_{hp}_{hh}")
                    nc.tensor.matmul(
                        rs[:], ones_p[:], sums[:, hh, :],
                        start=True, stop=True)
                    rinv = rvp.tile([1, 512], F32, tag="rv",
                                    name=f"rv{qc}_{hp}_{hh}")
                    nc.vector.reciprocal(rinv[:], rs[:])
                    rb = rbp.tile([128, 512], F32, tag="rb",
                                  name=f"rb{qc}_{hp}_{hh}")
                    nc.gpsimd.partition_broadcast(rb[:], rinv[:])
                    nc.vector.tensor_tensor(
                        o_sb[:, hh, :], pv[hh][:], rb[:], op=alu_mult)
                nc.sync.dma_start(
                    out=out_d[hp * 256:(hp + 1) * 256,
                              qc * 512:(qc + 1) * 512].rearrange(
                        "(h p) q -> p h q", p=128),
                    in_=o_sb[:])
        # emit any leftover fillers (shouldn't happen)
        while fillers:
            fillers.pop()[2]()


def build_nc():
    # Bacc (not raw Bass): its finalize passes split multi-sem waits
    # (move_matmul_waits_to_ldweights / generate_event_semaphores) to meet the
    # 1-wait-per-instruction hardware constraint walrus enforces.
    nc = bacc.Bacc("TRN2", target_bir_lowering=False)
    # xb is x[b] pre-transposed on the host and packed by t-chunk:
    # [tchunk, ci, c%128, t] fp16
    xb = nc.declare_dram_parameter("xb", [NQC, NCT, 128, 512], FP16, isOutput=False)
    wqt = nc.declare_dram_parameter("wqt", [C, DG], FP16, isOutput=False)
    wkt = nc.declare_dram_parameter("wkt", [C, DKV], FP16, isOutput=False)
    wvt = nc.declare_dram_parameter("wvt", [C, DKV], FP16, isOutput=False)
    # out is stored [d, t]; the host transposes back during assemble
    out_d = nc.declare_dram_parameter("out", [DG, T], F32, isOutput=True)
    with tile.TileContext(nc) as tc:
        _body(tc, xb, wqt, wkt, wvt, out_d)
    nc.compile()
    return nc


def make_in_maps(x, Wq, Wk, Wv):
    f16 = np.float16
    in_maps = []
    for b in range(B):
        xT = np.ascontiguousarray(x[b].T).astype(f16)        # [C, T]
        xb4 = np.ascontiguousarray(
            xT.reshape(NCT, 128, NQC, 512).transpose(2, 0, 1, 3))
        for g in range(GROUPS):
            in_maps.append({
                "xb": xb4,
                "wqt": np.ascontiguousarray(Wq[g * DG:(g + 1) * DG].T).astype(f16),
                "wkt": np.ascontiguousarray(Wk[g * DKV:(g + 1) * DKV].T).astype(f16),
                "wvt": np.ascontiguousarray(Wv[g * DKV:(g + 1) * DKV].T).astype(f16),
            })
    return in_maps


def assemble(results):
    out = np.empty((B, T, C), np.float32)
    for i, res in enumerate(results):
        b, g = divmod(i, GROUPS)
        out[b, :, g * DG:(g + 1) * DG] = res["out"].T
    return out


def run(x, Wq, Wk, Wv, **spmd_kwargs):
    nc = build_nc()
    in_maps = make_in_maps(x, Wq, Wk, Wv)
    return run_bass_kernel_spmd(nc, in_maps, list(range(8)), **spmd_kwargs)


def kernel(x, Wq, Wk, Wv):
    return assemble(run(x, Wq, Wk, Wv).results)


# revision 8
# speedup vs baseline: 1.4078x; 1.2589x over previous
"""GQA (16 query heads, 4 KV groups) forward kernel for 8 Trainium2 NeuronCores.

Sharding: core = (batch b in 0..1) x (kv-group g in 0..3).  Each core owns one
batch element and one whole KV group (4 query heads), computing the output
slice out[b, :, g*512:(g+1)*512].

Per-core plan (all matmul inputs fp16, fp32 PSUM accumulation):
  - x^T arrives from DRAM packed by t-chunk so K/V projections start as soon
    as the first 2.1MB chunk lands (DMA order: wk, wv, x0, wq, x1..x3).
  - K^T produced directly ([d, t], stationary Wk); V produced in natural
    layout ([t, d]) via x-stationary matmuls - no PE transposes anywhere.
  - Attention in transposed-score layout, two heads per pass so ACT exp and
    DVE sum-adds run on paired [128, 2, 512] tiles (half the instruction
    overhead).  Causal mask via gpsimd affine_select; exp restricted to the
    unmasked column range on diagonal blocks.
  - Softmax denominators via ones-vector matmul (partition reduction on PE),
    reciprocal on DVE, broadcast via gpsimd partition_broadcast, one DVE
    multiply to normalize - output stays in [d, q] layout and the host
    transposes it back during assemble.
  - Q^T chunks and later K/V projection chunks are interleaved as "filler"
    PE work inside the attention kb-loops so the PE never waits on ACT.
"""

import sys

if "/opt/trn_rl_repo" not in sys.path:
    sys.path.insert(0, "/opt/trn_rl_repo")

import numpy as np

import concourse.bass as bass
import concourse.mybir as mybir
import concourse.tile as tile
from concourse import bacc
from concourse.bass_utils import run_bass_kernel_spmd

B, T, C = 2, 2048, 2048
HEADS, GROUPS = 16, 4
HD = C // HEADS          # 128 head dim
H2G = HEADS // GROUPS    # 4 query heads per group
DG = H2G * HD            # 512 output cols per core
DKV = HD                 # 128 kv dim per group
NCT = C // 128           # 16 contraction tiles
NQC = T // 512           # 4 query chunks (= t chunks)
NKB = T // 128           # 16 key blocks
SCALE = HD ** -0.5

F32 = mybir.dt.float32
FP16 = mybir.dt.float16


def _body(tc, xb, wqt, wkt, wvt, out_d):
    nc = tc.nc
    act_exp = mybir.ActivationFunctionType.Exp
    is_ge = mybir.AluOpType.is_ge
    alu_mult = mybir.AluOpType.mult

    with (
        tc.tile_pool(name="const", bufs=1) as cpool,
        tc.tile_pool(name="data", bufs=1) as data,
        tc.tile_pool(name="qt_sb", bufs=2) as qtsb,
        tc.tile_pool(name="ex_sb", bufs=6) as expool,
        tc.tile_pool(name="sum_sb", bufs=3) as sump,
        tc.tile_pool(name="o_sb", bufs=2) as outp,
        tc.tile_pool(name="rb_sb", bufs=4) as rbp,
        tc.tile_pool(name="pv_ps", bufs=1, space="PSUM") as pvp,
        tc.tile_pool(name="st_ps", bufs=2, space="PSUM") as stp,
        tc.tile_pool(name="mi_ps", bufs=2, space="PSUM") as mip,
    ):
        ones_m = cpool.tile([128, 128], FP16)
        nc.vector.memset(ones_m[:], 1.0)

        xT = data.tile([128, NQC, NCT, 512], FP16)  # [c%128, tchunk, ci, t]
        wq = data.tile([128, NCT, DG], FP16)        # Wq^T tiles [c%128, ci, d]
        wk = data.tile([128, NCT, DKV], FP16)
        wv = data.tile([128, NCT, DKV], FP16)
        kT = data.tile([128, NQC, 512], FP16)       # K^T: [d, tchunk, t]
        vn = data.tile([128, NKB, DKV], FP16)       # V natural: [t%128, kb, d]

        # ---- input DMAs, in priority order ----
        nc.sync.dma_start(
            out=wk[:], in_=wkt.rearrange("(ci p) d -> p ci d", p=128))
        nc.sync.dma_start(
            out=wv[:], in_=wvt.rearrange("(ci p) d -> p ci d", p=128))
        for ci in range(NCT):
            nc.sync.dma_start(out=xT[:, 0, ci, :], in_=xb[0, ci])
        for c4 in range(4):
            nc.sync.dma_start(
                out=wq[:, c4 * 4:(c4 + 1) * 4, :],
                in_=wqt.rearrange("(ci p) d -> p ci d", p=128)[
                    :, c4 * 4:(c4 + 1) * 4, :])
        for tcx in range(1, NQC):
            for ci in range(NCT):
                nc.sync.dma_start(out=xT[:, tcx, ci, :], in_=xb[tcx, ci])

        # ---- projection chunk emitters (each ~1-4us of PE work) ----
        def k_chunk(tcx):
            ps = mip.tile([128, 512], F32, tag="mi", name=f"kp{tcx}")
            for ci in range(NCT):
                nc.tensor.matmul(
                    ps[:], wk[:, ci, :], xT[:, tcx, ci, :],
                    start=(ci == 0), stop=(ci == NCT - 1))
            nc.scalar.copy(kT[:, tcx, :], ps[:])

        def v_chunk(tcx, tb):
            ps = mip.tile([128, 128], F32, tag="mi", name=f"vp{tcx}_{tb}")
            for ci in range(NCT):
                nc.tensor.matmul(
                    ps[:], xT[:, tcx, ci, tb * 128:(tb + 1) * 128],
                    wv[:, ci, :],
                    start=(ci == 0), stop=(ci == NCT - 1))
            nc.scalar.copy(vn[:, tcx * 4 + tb, :], ps[:])

        qt_tiles = {}

        def q_chunk(qc, h):
            if qc not in qt_tiles:
                qt_tiles[qc] = qtsb.tile(
                    [128, H2G, 512], FP16, tag="qt", name=f"qt{qc}")
            qt = qt_tiles[qc]
            ps = mip.tile([128, 512], F32, tag="mi", name=f"qp{qc}_{h}")
            for ci in range(NCT):
                nc.tensor.matmul(
                    ps[:], wq[:, ci, h * 128:(h + 1) * 128],
                    xT[:, qc, ci, :],
                    start=(ci == 0), stop=(ci == NCT - 1))
            nc.scalar.copy(qt[:, h, :], ps[:])
            return qt

        # filler queue: (stage, deadline_iter_within_pass(stage,0), emit_fn)
        fillers = []
        for s in range(1, NQC):
            for h in range(H2G):
                fillers.append((s, 0, lambda s=s, h=h: q_chunk(s, h)))
            fillers.append((s, max(0, 4 * s - 2), lambda s=s: k_chunk(s)))
            for tb in range(4):
                fillers.append(
                    (s, max(0, 4 * s + tb - 2), lambda s=s, tb=tb: v_chunk(s, tb)))
        fillers.reverse()  # pop() from the front

        def drain_fillers(stage, itr):
            while fillers and (fillers[-1][0], fillers[-1][1]) <= (stage, itr):
                fillers.pop()[2]()

        def pop_filler():
            if fillers:
                fillers.pop()[2]()

        # ---- startup: KV chunk 0, Q^T chunk 0 ----
        k_chunk(0)
        for tb in range(4):
            v_chunk(0, tb)
        for h in range(H2G):
            q_chunk(0, h)

        # ---- attention: two heads per pass ----
        for qc in range(NQC):
            drain_fillers(qc, 0)
            qt = qt_tiles[qc]
            nkb = 4 * qc + 4
            cadence = max(1, qc + 1)
            for hp in range(2):
                sums = sump.tile([128, 2, 512], FP16, tag="sums",
                                 name=f"sums{qc}_{hp}")
                pv = [
                    pvp.tile([128, 512], F32, tag=f"pv{hh}", name=f"pv{qc}_{hp}_{hh}")
                    for hh in range(2)
                ]
                for kb in range(nkb):
                    if hp == 0:
                        drain_fillers(qc, kb)
                    diag = kb >= 4 * qc
                    base = (kb - 4 * qc) * 128 if diag else 0
                    st = stp.tile([128, 2, 512], F32, tag="st",
                                  name=f"st{qc}_{hp}_{kb}")
                    kblk = kT[:, kb // 4, (kb % 4) * 128:(kb % 4 + 1) * 128]
                    for hh in range(2):
                        nc.tensor.matmul(
                            st[:, hh, :], kblk, qt[:, 2 * hp + hh, :],
                            start=True, stop=True)
                    ex = expool.tile([128, 2, 512], FP16, tag="ex",
                                     name=f"ex{qc}_{hp}_{kb}")
                    nc.scalar.activation(
                        ex[:, :, base:], st[:, :, base:], act_exp, scale=SCALE)
                    if diag:
                        # columns < base are fully masked; [base, base+128)
                        # is the triangular boundary strip; >= base+128 kept.
                        if base > 0:
                            nc.gpsimd.memset(ex[:, :, :base], 0.0)
                        for hh in range(2):
                            nc.gpsimd.affine_select(
                                out=ex[:, hh, base:base + 128],
                                in_=ex[:, hh, base:base + 128],
                                compare_op=is_ge,
                                fill=0.0,
                                base=0,
                                pattern=[[1, 128]],
                                channel_multiplier=-1,
                            )
                    if kb == 0:
                        nc.vector.tensor_copy(sums[:], ex[:])
                    else:
                        nc.vector.tensor_add(sums[:], sums[:], ex[:])
                    for hh in range(2):
                        nc.tensor.matmul(
                            pv[hh][:], vn[:, kb, :], ex[:, hh, :],
                            start=(kb == 0), stop=(kb == nkb - 1))
                    if (hp * nkb + kb) % cadence == 0:
                        pop_filler()
                # ---- wrap-up: denominators + normalize + store ----
                o_sb = outp.tile([128, 2, 512], F32, tag="o",
                                 name=f"o{qc}_{hp}")
                for hh in range(2):
                    # ones_m.T @ sums = softmax denominator replicated across
                    # all 128 partitions, in one matmul
                    den = mip.tile([128, 512], F32, tag="mi",
                                   name=f"den{qc}_{hp}_{hh}")
                    nc.tensor.matmul(
                        den[:], ones_m[:], sums[:, hh, :],
                        start=True, stop=True)
                    rb = rbp.tile([128, 512], F32, tag="rb",
                                  name=f"rb{qc}_{hp}_{hh}")
                    nc.vector.reciprocal_approx_fast(rb[:], den[:])
                    nc.vector.tensor_tensor(
                        o_sb[:, hh, :], pv[hh][:], rb[:], op=alu_mult)
                nc.sync.dma_start(
                    out=out_d[hp * 256:(hp + 1) * 256,
                              qc * 512:(qc + 1) * 512].rearrange(
                        "(h p) q -> p h q", p=128),
                    in_=o_sb[:])
        # emit any leftover fillers (shouldn't happen)
        while fillers:
            fillers.pop()[2]()


def build_nc():
    # Bacc (not raw Bass): its finalize passes split multi-sem waits
    # (move_matmul_waits_to_ldweights / generate_event_semaphores) to meet the
    # 1-wait-per-instruction hardware constraint walrus enforces.
    nc = bacc.Bacc("TRN2", target_bir_lowering=False)
    # xb is x[b] pre-transposed on the host and packed by t-chunk:
    # [tchunk, ci, c%128, t] fp16
    xb = nc.declare_dram_parameter("xb", [NQC, NCT, 128, 512], FP16, isOutput=False)
    wqt = nc.declare_dram_parameter("wqt", [C, DG], FP16, isOutput=False)
    wkt = nc.declare_dram_parameter("wkt", [C, DKV], FP16, isOutput=False)
    wvt = nc.declare_dram_parameter("wvt", [C, DKV], FP16, isOutput=False)
    # out is stored [d, t]; the host transposes back during assemble
    out_d = nc.declare_dram_parameter("out", [DG, T], F32, isOutput=True)
    with tile.TileContext(nc) as tc:
        _body(tc, xb, wqt, wkt, wvt, out_d)
    nc.compile()
    return nc


def make_in_maps(x, Wq, Wk, Wv):
    f16 = np.float16
    in_maps = []
    for b in range(B):
        xT = np.ascontiguousarray(x[b].T).astype(f16)        # [C, T]
        xb4 = np.ascontiguousarray(
            xT.reshape(NCT, 128, NQC, 512).transpose(2, 0, 1, 3))
        for g in range(GROUPS):
            in_maps.append({
                "xb": xb4,
                "wqt": np.ascontiguousarray(Wq[g * DG:(g + 1) * DG].T).astype(f16),
                "wkt": np.ascontiguousarray(Wk[g * DKV:(g + 1) * DKV].T).astype(f16),
                "wvt": np.ascontiguousarray(Wv[g * DKV:(g + 1) * DKV].T).astype(f16),
            })
    return in_maps


def assemble(results):
    out = np.empty((B, T, C), np.float32)
    for i, res in enumerate(results):
        b, g = divmod(i, GROUPS)
        out[b, :, g * DG:(g + 1) * DG] = res["out"].T
    return out


def run(x, Wq, Wk, Wv, **spmd_kwargs):
    nc = build_nc()
    in_maps = make_in_maps(x, Wq, Wk, Wv)
    return run_bass_kernel_spmd(nc, in_maps, list(range(8)), **spmd_kwargs)


def kernel(x, Wq, Wk, Wv):
    return assemble(run(x, Wq, Wk, Wv).results)
